# revision 18
# baseline (speedup 1.0000x reference)
"""Expert-parallel MoE MLP kernel for Trainium2 (8 NeuronCores).

Problem: x[B=2,S=1024,H=1024] f32, expert_indices[B,S] int, 16 experts,
gate/up_proj[E,H,I], down_proj[E,I,H] (H=I=1024):
    out[n] = silu(x_n @ Wg[e_n]) * (x_n @ Wu[e_n]) @ Wd[e_n].T

Sharding: expert parallelism - core c owns experts {2c, 2c+1}. The host
groups tokens by expert (the "all-to-all dispatch" runs on host since the
kernel contract is full-input -> full-output), pads each expert's token
block to a 16-multiple capacity, and each core runs dense per-expert GEMMs.

All operands are bf16 (rel err ~4e-3 vs the 2e-2 gate), which halves the
mandatory weight traffic to 12 MB/core - the roofline (~36 us at 360 GB/s
across the 16 DMA engines).

The device program is RAW bass (no Tile framework) with 9 hand-rolled
counting semaphores. The Tile layer costs ~10 us here: a ~250-semaphore
teardown (~6 us of serialized clears), two all-engine barrier phases, and
an 8-semaphore HWDGE rotation that couples the weight stream to compute
progress and stalls it whenever a chunk's consumers run late. Raw bass:
  - all 25 weight-queue configs issue immediately on the SP queue with a
    single counting completion semaphore (+16/DMA); the stream never
    waits on compute
  - PE chases the stream h-outer (uniform 0.5 MB chunks, 4 KB runs),
    at the real token width (bf16 matmul is 1 cycle/row at any width)
  - PSUM is partitioned gate 3 banks / up 3 / down 2 (3 accumulation
    regions per bank), so gate->up->down never serialize on banks
  - xt[e1] rides the SP FIFO right before expert 1's weights; xt[e0]
    and the 8 output stores ride the Act queue in parallel
"""

import math

import numpy as np

E = 16
H = 1024
HT = 8           # H / 128 partition tiles
N_CORES = 8
EPC = E // N_CORES   # experts per core
NS = 4           # DMA chunks per projection (0.5 MB each)
HH = HT // NS    # h-tiles per gate/up chunk

_NC_CACHE = {}


def _build_nc_raw(pio: int, act: str = "Silu"):
    """Raw-bass SPMD program. pio: padded token count, multiple of 16,
    <= 160 (3 PSUM accumulation regions per 2 KB bank)."""
    from concourse import bacc, mybir
    from concourse.bass import ts

    f32 = mybir.dt.float32
    bf16 = mybir.dt.bfloat16
    SILU = getattr(mybir.ActivationFunctionType, act)
    assert 3 * pio * 4 <= 2048

    nc = bacc.Bacc("TRN2", target_bir_lowering=False, debug=False,
                   num_devices=N_CORES)
    w = nc.dram_tensor("w", [EPC, 2, 128, HT, H], bf16, kind="ExternalInput")
    wd = nc.dram_tensor("wd", [EPC, 128, NS, HT, 256], bf16,
                        kind="ExternalInput")
    xt = nc.dram_tensor("xt", [EPC, 128, HT, pio], bf16, kind="ExternalInput")
    out = nc.dram_tensor("out", [EPC, 128, HT, pio], bf16,
                         kind="ExternalOutput")

    import contextlib
    with contextlib.ExitStack() as st:
        s_ws = [st.enter_context(nc.semaphore(f"s_w{i}")) for i in range(12)]
        s_x = st.enter_context(nc.semaphore("s_x"))   # xt[e0] (+16)
        s_g = st.enter_context(nc.semaphore("s_g"))   # gate region done (PE)
        s_u = st.enter_context(nc.semaphore("s_u"))   # up region done (PE)
        s_s = st.enter_context(nc.semaphore("s_s"))   # silu done (Act)
        s_m = st.enter_context(nc.semaphore("s_m"))   # inter mul done (DVE)
        s_d = st.enter_context(nc.semaphore("s_d"))   # down region done (PE)
        s_c = st.enter_context(nc.semaphore("s_c"))   # out copy done (DVE)
        s_o = st.enter_context(nc.semaphore("s_o"))   # out stores (+16 each)
        s_o2 = st.enter_context(nc.semaphore("s_o2"))  # final out on SP queue
        wgu = st.enter_context(
            nc.sbuf_tensor("wgu", [128, 2 * NS * EPC, HH, H], bf16))
        wdn = st.enter_context(
            nc.sbuf_tensor("wdn", [128, NS * EPC, HT, 256], bf16))
        x_sb = st.enter_context(
            nc.sbuf_tensor("x_sb", [128, EPC, HT, pio], bf16))
        g_sb = st.enter_context(
            nc.sbuf_tensor("g_sb", [128, EPC, HT, pio], f32))
        i_sb = st.enter_context(
            nc.sbuf_tensor("i_sb", [128, EPC, HT, pio], bf16))
        o_sb = st.enter_context(
            nc.sbuf_tensor("o_sb", [128, EPC, HT, pio], bf16))
        # one 8-bank PSUM pool; only one accumulation group may be open
        # per bank, so gate/up/down reuse banks with explicit WAR waits
        p8 = st.enter_context(nc.psum_tensor("p8", [128, 8, 512], f32))

        # weight-queue completion: 8 rotating counting semaphores (adjacent
        # queue entries can fuse, so each entry needs its own wait level)
        def w_dma(sync, dst, src, k):
            # HW sem protocol: before reusing a rotated completion sem
            # level, the issuer must wait for the previous level (keeps 12
            # chunks = 6 MB in flight - completion-paced, not compute-paced)
            if k >= 12:
                sync.wait_ge(s_ws[k % 12], 16 * (k // 12))
            sync.dma_start(dst, src).then_inc(s_ws[k % 12], 16)

        def w_wait(eng, k):
            eng.wait_ge(s_ws[k % 12], 16 * (k // 12 + 1))
        def reg(i):
            return p8[:, i, 0:pio]

        def dreg(q, jl):
            return p8[:, 2 * (q % 2) + jl, 0:pio]

        # sync-queue entry index -> s_w target is 16*(entry+1)
        GQ = lambda e, q: e * 13 + q           # gate chunks
        UQ = lambda e, q: e * 13 + 4 + q       # up chunks
        DQ = lambda e, q: e * 13 + 8 + q       # down chunks
        XT1 = 12                               # xt[e1] rides before e1 weights

        with nc.Block() as block:

            @block.sync
            def _(sync):
                for e in range(EPC):
                    if e > 0:
                        w_dma(sync, x_sb[:, e], xt[e], XT1)
                    for proj in range(2):
                        for q in range(NS):
                            c = (e * 2 + proj) * NS + q
                            k = (GQ, UQ)[proj](e, q)
                            w_dma(sync, wgu[:, c],
                                  w[e, proj, :, ts(q, HH), :], k)
                    for q in range(NS):
                        w_dma(sync, wdn[:, e * NS + q], wd[e, :, q],
                              DQ(e, q))
                # final j-tile ships from the (by now idle) SP queue so it
                # doesn't wait behind the Act queue's previous store
                sync.wait_ge(s_c, 8 * EPC)
                sync.dma_start(out[EPC - 1, :, HT - 1:HT, :],
                               o_sb[:, EPC - 1, HT - 1:HT]).then_inc(s_o2, 16)

            @block.tensor
            def _(tensor):
                tensor.wait_ge(s_x, 16)
                for e in range(EPC):
                    if e > 0:
                        w_wait(tensor, XT1)
                        tensor.wait_ge(s_m, 8 * e)   # banks 4-7: muls done
                        tensor.wait_ge(s_c, 8 * e)   # banks 0-3: copies done
                    # gate, h-outer, chasing chunk arrival
                    for h in range(HT):
                        if h % HH == 0:
                            w_wait(tensor, GQ(e, h // HH))
                        c = (e * 2 + 0) * NS + h // HH
                        for i in range(HT):
                            mm = tensor.matmul(
                                reg(i), wgu[:, c, h % HH, ts(i, 128)],
                                x_sb[:, e, h],
                                start=(h == 0), stop=(h == HT - 1))
                            if h == HT - 1:
                                mm.then_inc(s_g)
                    # up (reuses gate's banks; region i waits silu[i])
                    for h in range(HT):
                        if h % HH == 0:
                            w_wait(tensor, UQ(e, h // HH))
                        c = (e * 2 + 1) * NS + h // HH
                        for i in range(HT):
                            if h == 0:
                                tensor.wait_ge(s_s, 8 * e + i + 1)
                            mm = tensor.matmul(
                                reg(i), wgu[:, c, h % HH, ts(i, 128)],
                                x_sb[:, e, h],
                                start=(h == 0), stop=(h == HT - 1))
                            if h == HT - 1:
                                mm.then_inc(s_u)
                    # down, j-sliced chunks
                    tensor.wait_ge(s_m, 8 * (e + 1))  # inter ready
                    for q in range(NS):
                        w_wait(tensor, DQ(e, q))
                        if e * NS + q >= 2:           # pd bank ping-pong WAR
                            tensor.wait_ge(s_c, 2 * (e * NS + q) - 2)
                        for jl in range(2):
                            for k in range(HT):
                                mm = tensor.matmul(
                                    dreg(q, jl),
                                    wdn[:, e * NS + q, k, ts(jl, 128)],
                                    i_sb[:, e, k],
                                    start=(k == 0), stop=(k == HT - 1))
                                if k == HT - 1:
                                    mm.then_inc(s_d)

            @block.scalar
            def _(scalar):
                scalar.dma_start(x_sb[:, 0], xt[0]).then_inc(s_x, 16)
                for e in range(EPC):
                    for i in range(HT):
                        scalar.wait_ge(s_g, 8 * e + i + 1)
                        scalar.activation(g_sb[:, e, i], reg(i),
                                          SILU).then_inc(s_s)
                    for q in range(NS):
                        last = e == EPC - 1 and q == NS - 1
                        scalar.wait_ge(s_c, 8 * e + 2 * q + (1 if last else 2))
                        j1 = 2 * q + (1 if last else 2)
                        scalar.dma_start(
                            out[e, :, 2 * q:j1, :],
                            o_sb[:, e, 2 * q:j1]).then_inc(s_o, 16)
                scalar.wait_ge(s_o, 16 * 4 * EPC)    # drain output stores
                scalar.wait_ge(s_o2, 16)

            @block.vector
            def _(vector):
                for e in range(EPC):
                    for i in range(HT):
                        vector.wait_ge(s_s, 8 * e + i + 1)
                        vector.wait_ge(s_u, 8 * e + i + 1)
                        vector.tensor_mul(i_sb[:, e, i], g_sb[:, e, i],
                                          reg(i)).then_inc(s_m)
                    for j in range(HT):
                        vector.wait_ge(s_d, 8 * e + j + 1)
                        vector.tensor_copy(o_sb[:, e, j],
                                           dreg(j // 2, j % 2)).then_inc(s_c)

    nc.compile()
    return nc


def _get_nc(pio: int):
    if pio not in _NC_CACHE:
        _NC_CACHE[pio] = _build_nc_raw(pio)
    return _NC_CACHE[pio]


_ROUND_CAP = 160          # max tokens/expert per round (3 PSUM regions/bank)


def _kernel_once(x, expert_indices, gate_proj, up_proj, down_proj):
    import ml_dtypes
    from concourse.bass_utils import run_bass_kernel_spmd

    bf16 = np.dtype(ml_dtypes.bfloat16)
    x = np.ascontiguousarray(x, dtype=np.float32)
    b, s, h = x.shape
    assert (h, gate_proj.shape) == (H, (E, H, H)), (x.shape, gate_proj.shape)

    n = b * s
    xf = x.reshape(n, h)
    idx = np.asarray(expert_indices).reshape(n).astype(np.int64)

    order = np.argsort(idx, kind="stable")       # token ids grouped by expert
    counts = np.bincount(idx, minlength=E)
    starts = np.zeros(E + 1, dtype=np.int64)
    np.cumsum(counts, out=starts[1:])
    maxc = int(counts.max())
    assert maxc <= _ROUND_CAP
    pio = max(16, 16 * math.ceil(maxc / 16))

    # per-core weight packing (bf16, partition-major)
    wr = np.stack([gate_proj, up_proj], axis=1).astype(bf16) \
        .reshape(N_CORES, EPC, 2, HT, 128, H).transpose(0, 1, 2, 4, 3, 5)
    wdr = np.ascontiguousarray(down_proj.transpose(0, 2, 1)).astype(bf16) \
        .reshape(N_CORES, EPC, HT, 128, NS, 256).transpose(0, 1, 3, 4, 2, 5)
    in_maps = []
    tok_ids = []
    for c in range(N_CORES):
        xt_c = np.zeros((EPC, H, pio), dtype=np.float32)
        toks = []
        for le in range(EPC):
            e = c * EPC + le
            te = order[starts[e]:starts[e + 1]]
            toks.append(te)
            xt_c[le, :, :len(te)] = xf[te].T
        tok_ids.append(toks)
        in_maps.append({
            "w": np.ascontiguousarray(wr[c]),
            "wd": np.ascontiguousarray(wdr[c]),
            "xt": xt_c.astype(bf16).reshape(EPC, HT, 128, pio)
                  .transpose(0, 2, 1, 3).copy(),
        })

    nc = _get_nc(pio)
    res = run_bass_kernel_spmd(nc, in_maps, core_ids=list(range(N_CORES)))

    out = np.empty((n, h), dtype=np.float32)
    for c in range(N_CORES):
        o = res.results[c]["out"]                # [EPC, 128, HT, pio] bf16
        for le in range(EPC):
            te = tok_ids[c][le]
            oe = np.asarray(o[le]).astype(np.float32) \
                .transpose(1, 0, 2).reshape(h, pio)      # [H, pio]
            out[te] = oe[:, :len(te)].T
    return out.reshape(b, s, h)


def kernel(x, expert_indices, gate_proj, up_proj, down_proj):
    """Full-input -> full-output entry point.

    Tokens-per-expert above _ROUND_CAP (pathological skew; PSUM bound)
    are handled by running the device kernel in multiple rounds over
    disjoint token slices - outputs are per-token independent."""
    idx = np.asarray(expert_indices)
    counts = np.bincount(idx.reshape(-1).astype(np.int64), minlength=E)
    if counts.max() <= _ROUND_CAP:
        return _kernel_once(x, expert_indices, gate_proj, up_proj, down_proj)

    b, s, h = x.shape
    n = b * s
    xf = np.ascontiguousarray(x, dtype=np.float32).reshape(n, h)
    idxf = idx.reshape(n).astype(np.int64)
    order = np.argsort(idxf, kind="stable")
    starts = np.zeros(E + 1, dtype=np.int64)
    np.cumsum(np.bincount(idxf, minlength=E), out=starts[1:])
    out = np.empty((n, h), dtype=np.float32)
    rounds = math.ceil(counts.max() / _ROUND_CAP)
    for r in range(rounds):
        sel = np.concatenate([
            order[starts[e] + r * _ROUND_CAP:
                  min(starts[e] + (r + 1) * _ROUND_CAP, starts[e + 1])]
            for e in range(E)])
        if not len(sel):
            continue
        xr = xf[sel].reshape(1, len(sel), h)
        ir = idxf[sel].reshape(1, len(sel))
        out[sel] = _kernel_once(
            xr, ir, gate_proj, up_proj, down_proj).reshape(len(sel), h)
    return out.reshape(b, s, h)


# revision 19
# speedup vs baseline: 1.1492x; 1.1492x over previous
"""Expert-parallel MoE MLP kernel for Trainium2 (8 NeuronCores).

Problem: x[B=2,S=1024,H=1024] f32, expert_indices[B,S] int, 16 experts,
gate/up_proj[E,H,I], down_proj[E,I,H] (H=I=1024):
    out[n] = silu(x_n @ Wg[e_n]) * (x_n @ Wu[e_n]) @ Wd[e_n].T

Sharding: expert parallelism - core c owns experts {2c, 2c+1}. The host
groups tokens by expert (the "all-to-all dispatch" runs on host since the
kernel contract is full-input -> full-output), pads each expert's token
block to a 16-multiple capacity, and each core runs dense per-expert GEMMs.

All operands are bf16 (rel err ~4e-3 vs the 2e-2 gate), which halves the
mandatory weight traffic to 12 MB/core - the roofline (~36 us at 360 GB/s
across the 16 DMA engines).

The device program is RAW bass (no Tile framework) with 9 hand-rolled
counting semaphores. The Tile layer costs ~10 us here: a ~250-semaphore
teardown (~6 us of serialized clears), two all-engine barrier phases, and
an 8-semaphore HWDGE rotation that couples the weight stream to compute
progress and stalls it whenever a chunk's consumers run late. Raw bass:
  - all 25 weight-queue configs issue immediately on the SP queue with a
    single counting completion semaphore (+16/DMA); the stream never
    waits on compute
  - PE chases the stream h-outer (uniform 0.5 MB chunks, 4 KB runs),
    at the real token width (bf16 matmul is 1 cycle/row at any width)
  - PSUM is partitioned gate 3 banks / up 3 / down 2 (3 accumulation
    regions per bank), so gate->up->down never serialize on banks
  - xt[e1] rides the SP FIFO right before expert 1's weights; xt[e0]
    and the 8 output stores ride the Act queue in parallel
"""

import math

import numpy as np

E = 16
H = 1024
HT = 8           # H / 128 partition tiles
N_CORES = 8
EPC = E // N_CORES   # experts per core
NS = 4           # DMA chunks per projection (0.5 MB each)
HH = HT // NS    # h-tiles per gate/up chunk

_NC_CACHE = {}


def _build_nc_raw(pio: int, act: str = "Silu"):
    """Raw-bass SPMD program. pio: padded token count, multiple of 16,
    <= 160 (3 PSUM accumulation regions per 2 KB bank)."""
    from concourse import bacc, mybir
    from concourse.bass import ts

    f32 = mybir.dt.float32
    bf16 = mybir.dt.bfloat16
    SILU = getattr(mybir.ActivationFunctionType, act)
    assert 3 * pio * 4 <= 2048

    nc = bacc.Bacc("TRN2", target_bir_lowering=False, debug=False,
                   num_devices=N_CORES)
    w = nc.dram_tensor("w", [EPC, 2, 128, HT, H], bf16, kind="ExternalInput")
    wd = nc.dram_tensor("wd", [EPC, 128, NS, HT, 256], bf16,
                        kind="ExternalInput")
    xt = nc.dram_tensor("xt", [EPC, 128, HT, pio], bf16, kind="ExternalInput")
    out = nc.dram_tensor("out", [EPC, 128, HT, pio], bf16,
                         kind="ExternalOutput")

    import contextlib
    with contextlib.ExitStack() as st:
        s_ws = [st.enter_context(nc.semaphore(f"s_w{i}")) for i in range(8)]
        s_x = st.enter_context(nc.semaphore("s_x"))   # xt[e0] (+16)
        s_g = st.enter_context(nc.semaphore("s_g"))   # gate region done (PE)
        s_u = st.enter_context(nc.semaphore("s_u"))   # up region done (PE)
        s_s = st.enter_context(nc.semaphore("s_s"))   # silu done (Act)
        s_m = st.enter_context(nc.semaphore("s_m"))   # inter mul done (DVE)
        s_d = st.enter_context(nc.semaphore("s_d"))   # down region done (PE)
        s_c = st.enter_context(nc.semaphore("s_c"))   # out copy done (DVE)
        s_o = st.enter_context(nc.semaphore("s_o"))   # out stores (+16 each)
        s_o2 = st.enter_context(nc.semaphore("s_o2"))  # final out on SP queue
        wgu = st.enter_context(
            nc.sbuf_tensor("wgu", [128, 2 * NS * EPC, HH, H], bf16))
        wdn = st.enter_context(
            nc.sbuf_tensor("wdn", [128, NS * EPC, HT, 256], bf16))
        x_sb = st.enter_context(
            nc.sbuf_tensor("x_sb", [128, EPC, HT, pio], bf16))
        g_sb = st.enter_context(
            nc.sbuf_tensor("g_sb", [128, EPC, HT, pio], f32))
        i_sb = st.enter_context(
            nc.sbuf_tensor("i_sb", [128, EPC, HT, pio], bf16))
        o_sb = st.enter_context(
            nc.sbuf_tensor("o_sb", [128, EPC, HT, pio], bf16))
        # one 8-bank PSUM pool; only one accumulation group may be open
        # per bank, so gate/up/down reuse banks with explicit WAR waits
        p8 = st.enter_context(nc.psum_tensor("p8", [128, 8, 512], f32))

        # weight-queue completion: 8 rotating counting semaphores (adjacent
        # queue entries can fuse, so each entry needs its own wait level)
        def w_dma(sync, dst, src, k):
            # HW sem protocol: before reusing a rotated completion sem
            # level, the issuer must wait for the previous level (keeps 8
            # chunks = 4 MB in flight - completion-paced, not compute-paced)
            if k >= 8:
                sync.wait_ge(s_ws[k % 8], 16 * (k // 8))
            sync.dma_start(dst, src).then_inc(s_ws[k % 8], 16)

        def w_wait(eng, k):
            eng.wait_ge(s_ws[k % 8], 16 * (k // 8 + 1))
        def reg(i):
            return p8[:, i, 0:pio]

        def dreg(q, jl):
            return p8[:, 2 * (q % 2) + jl, 0:pio]

        # sync-queue entry index -> s_w target is 16*(entry+1)
        GQ = lambda e, q: e * 13 + q           # gate chunks
        UQ = lambda e, q: e * 13 + 4 + q       # up chunks
        DQ = lambda e, q: e * 13 + 8 + q       # down chunks
        XT1 = 12                               # xt[e1] rides before e1 weights

        with nc.Block() as block:

            @block.sync
            def _(sync):
                for e in range(EPC):
                    if e > 0:
                        w_dma(sync, x_sb[:, e], xt[e], XT1)
                    for proj in range(2):
                        for q in range(NS):
                            c = (e * 2 + proj) * NS + q
                            k = (GQ, UQ)[proj](e, q)
                            w_dma(sync, wgu[:, c],
                                  w[e, proj, :, ts(q, HH), :], k)
                    for q in range(NS):
                        w_dma(sync, wdn[:, e * NS + q], wd[e, :, q],
                              DQ(e, q))
                # final j-tile ships from the (by now idle) SP queue so it
                # doesn't wait behind the Act queue's previous store
                sync.wait_ge(s_c, 8 * EPC)
                sync.dma_start(out[EPC - 1, :, HT - 1:HT, :],
                               o_sb[:, EPC - 1, HT - 1:HT]).then_inc(s_o2, 16)

            @block.tensor
            def _(tensor):
                tensor.wait_ge(s_x, 16)
                for e in range(EPC):
                    if e > 0:
                        w_wait(tensor, XT1)
                        tensor.wait_ge(s_m, 8 * e)   # banks 4-7: muls done
                        tensor.wait_ge(s_c, 8 * e)   # banks 0-3: copies done
                    # gate, h-outer, chasing chunk arrival
                    for h in range(HT):
                        if h % HH == 0:
                            w_wait(tensor, GQ(e, h // HH))
                        c = (e * 2 + 0) * NS + h // HH
                        for i in range(HT):
                            mm = tensor.matmul(
                                reg(i), wgu[:, c, h % HH, ts(i, 128)],
                                x_sb[:, e, h],
                                start=(h == 0), stop=(h == HT - 1))
                            if h == HT - 1:
                                mm.then_inc(s_g)
                    # up (reuses gate's banks; region i waits silu[i])
                    for h in range(HT):
                        if h % HH == 0:
                            w_wait(tensor, UQ(e, h // HH))
                        c = (e * 2 + 1) * NS + h // HH
                        for i in range(HT):
                            if h == 0:
                                tensor.wait_ge(s_s, 8 * e + i + 1)
                            mm = tensor.matmul(
                                reg(i), wgu[:, c, h % HH, ts(i, 128)],
                                x_sb[:, e, h],
                                start=(h == 0), stop=(h == HT - 1))
                            if h == HT - 1:
                                mm.then_inc(s_u)
                    # down, j-sliced chunks
                    tensor.wait_ge(s_m, 8 * (e + 1))  # inter ready
                    for q in range(NS):
                        w_wait(tensor, DQ(e, q))
                        if e * NS + q >= 2:           # pd bank ping-pong WAR
                            tensor.wait_ge(s_c, 2 * (e * NS + q) - 2)
                        for jl in range(2):
                            for k in range(HT):
                                mm = tensor.matmul(
                                    dreg(q, jl),
                                    wdn[:, e * NS + q, k, ts(jl, 128)],
                                    i_sb[:, e, k],
                                    start=(k == 0), stop=(k == HT - 1))
                                if k == HT - 1:
                                    mm.then_inc(s_d)

            @block.scalar
            def _(scalar):
                scalar.dma_start(x_sb[:, 0], xt[0]).then_inc(s_x, 16)
                for e in range(EPC):
                    for i in range(HT):
                        scalar.wait_ge(s_g, 8 * e + i + 1)
                        scalar.activation(g_sb[:, e, i], reg(i),
                                          SILU).then_inc(s_s)
                    for q in range(NS):
                        last = e == EPC - 1 and q == NS - 1
                        scalar.wait_ge(s_c, 8 * e + 2 * q + (1 if last else 2))
                        j1 = 2 * q + (1 if last else 2)
                        scalar.dma_start(
                            out[e, :, 2 * q:j1, :],
                            o_sb[:, e, 2 * q:j1]).then_inc(s_o, 16)
                scalar.wait_ge(s_o, 16 * 4 * EPC)    # drain output stores
                scalar.wait_ge(s_o2, 16)

            @block.vector
            def _(vector):
                for e in range(EPC):
                    for i in range(HT):
                        vector.wait_ge(s_s, 8 * e + i + 1)
                        vector.wait_ge(s_u, 8 * e + i + 1)
                        vector.tensor_mul(i_sb[:, e, i], g_sb[:, e, i],
                                          reg(i)).then_inc(s_m)
                    for j in range(HT):
                        vector.wait_ge(s_d, 8 * e + j + 1)
                        vector.tensor_copy(o_sb[:, e, j],
                                           dreg(j // 2, j % 2)).then_inc(s_c)

    nc.compile()
    return nc


def _get_nc(pio: int):
    if pio not in _NC_CACHE:
        _NC_CACHE[pio] = _build_nc_raw(pio)
    return _NC_CACHE[pio]


_ROUND_CAP = 160          # max tokens/expert per round (3 PSUM regions/bank)


def _kernel_once(x, expert_indices, gate_proj, up_proj, down_proj):
    import ml_dtypes
    from concourse.bass_utils import run_bass_kernel_spmd

    bf16 = np.dtype(ml_dtypes.bfloat16)
    x = np.ascontiguousarray(x, dtype=np.float32)
    b, s, h = x.shape
    assert (h, gate_proj.shape) == (H, (E, H, H)), (x.shape, gate_proj.shape)

    n = b * s
    xf = x.reshape(n, h)
    idx = np.asarray(expert_indices).reshape(n).astype(np.int64)

    order = np.argsort(idx, kind="stable")       # token ids grouped by expert
    counts = np.bincount(idx, minlength=E)
    starts = np.zeros(E + 1, dtype=np.int64)
    np.cumsum(counts, out=starts[1:])
    maxc = int(counts.max())
    assert maxc <= _ROUND_CAP
    pio = max(16, 16 * math.ceil(maxc / 16))

    # per-core weight packing (bf16, partition-major)
    wr = np.stack([gate_proj, up_proj], axis=1).astype(bf16) \
        .reshape(N_CORES, EPC, 2, HT, 128, H).transpose(0, 1, 2, 4, 3, 5)
    wdr = np.ascontiguousarray(down_proj.transpose(0, 2, 1)).astype(bf16) \
        .reshape(N_CORES, EPC, HT, 128, NS, 256).transpose(0, 1, 3, 4, 2, 5)
    in_maps = []
    tok_ids = []
    for c in range(N_CORES):
        xt_c = np.zeros((EPC, H, pio), dtype=np.float32)
        toks = []
        for le in range(EPC):
            e = c * EPC + le
            te = order[starts[e]:starts[e + 1]]
            toks.append(te)
            xt_c[le, :, :len(te)] = xf[te].T
        tok_ids.append(toks)
        in_maps.append({
            "w": np.ascontiguousarray(wr[c]),
            "wd": np.ascontiguousarray(wdr[c]),
            "xt": xt_c.astype(bf16).reshape(EPC, HT, 128, pio)
                  .transpose(0, 2, 1, 3).copy(),
        })

    nc = _get_nc(pio)
    res = run_bass_kernel_spmd(nc, in_maps, core_ids=list(range(N_CORES)))

    out = np.empty((n, h), dtype=np.float32)
    for c in range(N_CORES):
        o = res.results[c]["out"]                # [EPC, 128, HT, pio] bf16
        for le in range(EPC):
            te = tok_ids[c][le]
            oe = np.asarray(o[le]).astype(np.float32) \
                .transpose(1, 0, 2).reshape(h, pio)      # [H, pio]
            out[te] = oe[:, :len(te)].T
    return out.reshape(b, s, h)


def kernel(x, expert_indices, gate_proj, up_proj, down_proj):
    """Full-input -> full-output entry point.

    Tokens-per-expert above _ROUND_CAP (pathological skew; PSUM bound)
    are handled by running the device kernel in multiple rounds over
    disjoint token slices - outputs are per-token independent."""
    idx = np.asarray(expert_indices)
    counts = np.bincount(idx.reshape(-1).astype(np.int64), minlength=E)
    if counts.max() <= _ROUND_CAP:
        return _kernel_once(x, expert_indices, gate_proj, up_proj, down_proj)

    b, s, h = x.shape
    n = b * s
    xf = np.ascontiguousarray(x, dtype=np.float32).reshape(n, h)
    idxf = idx.reshape(n).astype(np.int64)
    order = np.argsort(idxf, kind="stable")
    starts = np.zeros(E + 1, dtype=np.int64)
    np.cumsum(np.bincount(idxf, minlength=E), out=starts[1:])
    out = np.empty((n, h), dtype=np.float32)
    rounds = math.ceil(counts.max() / _ROUND_CAP)
    for r in range(rounds):
        sel = np.concatenate([
            order[starts[e] + r * _ROUND_CAP:
                  min(starts[e] + (r + 1) * _ROUND_CAP, starts[e + 1])]
            for e in range(E)])
        if not len(sel):
            continue
        xr = xf[sel].reshape(1, len(sel), h)
        ir = idxf[sel].reshape(1, len(sel))
        out[sel] = _kernel_once(
            xr, ir, gate_proj, up_proj, down_proj).reshape(len(sel), h)
    return out.reshape(b, s, h)
